# revision 38
# baseline (speedup 1.0000x reference)
"""GCN layer kernel for Trainium2, batch-parallel across 8 NeuronCores.

out[b] = D^-1/2 (A[b]+I) D^-1/2 @ x[b] @ W.T + b_vec

Per-core dataflow (core c owns batch element c):
  - adj row-tiles [128, N] are loaded HBM->SBUF with an fp32->fp16 cast
    done inline by the SWDGE DMA path (gpsimd queue), so no engine pass
    is spent on casting and no fp32 staging buffer is needed.
  - Degrees: one DVE tensor_reduce per row-tile over the fp16 tile
    (fp32 accumulate), then d = Rsqrt(rowsum + 1) on the scalar engine.
  - The PE transposes each 128x128 block into a resident A^T buffer in
    SBUF (matmul needs the contraction index on the partition dim).
    Transpose-PSUM drains alternate between the scalar and vector
    engines so neither becomes the bottleneck.
  - The degree scaling is folded into x (x' = d*x) and the output
    (out = d * (...)), so adj_norm is never materialized.
  - Aggregation runs as PSUM-accumulated matmuls aggT[f, n] += x'_j^T AT_j,
    fired as soon as their row-tile dependencies are satisfied. The +I
    self-loop is folded in as x'_u^T @ I identity matmuls on the PE.
  - Warmup: a burst of regular (non-transpose) identity matmuls during
    the DMA fill phase lifts the HAM clock gate to K=8/8 early.
  - Tail: chunks 0..2 are closed (their j=15 steps fired) before chunk
    3's big burst so their drain/linear/write chains overlap it.
  - Tail: out[n,o] = d[n] * (aggT^T @ W^T)[n,o] + b[o], written out in
    2-tile batches alternating across both HWDGE queues.
"""

import numpy as np

B, N, F = 8, 2048, 128
P = 128                # partition tile / block size
NT = N // P            # 16 row tiles
NCHUNK = 512           # moving-dim chunk for the aggregation matmul
NCH = N // NCHUNK      # 4 chunks
TPC = NCHUNK // P      # row tiles per chunk
WARMUP = 32            # regular matmuls at start to lift the HAM clock gate
AGG_CAP = 12           # aggregation matmuls fired per row-tile

_PROGRAM_CACHE = {}


def _build_program(agg_dtype_name="float16", bufs_a=10, warm=WARMUP, cap=AGG_CAP):
    import concourse.bacc as bacc
    import concourse.bass as bass
    import concourse.tile as tile
    from concourse import mybir
    from concourse.masks import make_identity

    f32 = mybir.dt.float32
    agg_dt = getattr(mybir.dt, agg_dtype_name)

    nc = bacc.Bacc(
        "TRN2",
        target_bir_lowering=False,
        debug=False,
        num_devices=B,
        # no cross-core branching: per-core data arrives via in_maps, so
        # skip the partition-id register load sequence in the preamble
        enable_partition_id=False,
    )
    x_d = nc.dram_tensor("x", [N, F], f32, kind="ExternalInput")
    a_d = nc.dram_tensor("adj", [N, N], f32, kind="ExternalInput")
    w_d = nc.dram_tensor("W", [F, F], f32, kind="ExternalInput")
    b_d = nc.dram_tensor("b", [F], f32, kind="ExternalInput")
    o_d = nc.dram_tensor("out", [N, F], f32, kind="ExternalOutput")

    with tile.TileContext(nc) as tc:
        with (
            tc.tile_pool(name="singles", bufs=1) as singles,
            tc.tile_pool(name="a_cast", bufs=bufs_a) as a_cast,
            tc.tile_pool(name="x_pool", bufs=1) as x_pool,
            tc.tile_pool(name="small", bufs=6) as small,
            tc.tile_pool(name="tp_psum", bufs=4, space="PSUM") as tp_psum,
            tc.tile_pool(name="agg_psum", bufs=1, space="PSUM") as agg_psum,
        ):
            # prefetch: the first adj tiles are dispatched before ANY
            # other gpsimd work (make_identity runs on gpsimd and would
            # otherwise delay the first SWDGE load by ~1.5us); later
            # tiles are issued from inside the loop (slot-gated). The
            # adj stream is engine-paced, not DMA-bound, so small 1 MiB
            # tiles win: the first tile lands ~3us sooner than a 2 MiB
            # chunk would, and the whole pipeline shifts with it.
            PREF = 4
            SPLIT = {0, 1}   # tiles loaded as two half-DMAs (earlier start)
            a_tiles = {}

            def load_adj(t):
                # fp32 HBM -> fp16 SBUF, cast inline on the SWDGE path.
                # The first tiles are split into two half-row DMAs so the
                # degree pass and the first transposes start ~4us sooner.
                a_c = a_cast.tile([P, N], agg_dt, name="a_c")
                if t in SPLIT:
                    h = N // 2
                    nc.gpsimd.dma_start(a_c[:, :h], a_d[P * t : P * (t + 1), :h])
                    nc.gpsimd.dma_start(a_c[:, h:], a_d[P * t : P * (t + 1), h:])
                else:
                    nc.gpsimd.dma_start(a_c, a_d[P * t : P * (t + 1), :])
                a_tiles[t] = a_c

            for t in range(PREF):
                load_adj(t)

            # preload the Sqrt activation table immediately (no DMA deps:
            # the input is a DVE memset) -- avoids a ~2.7us ACT_TABLE_LOAD
            # stall at the first degree compute
            pre_in = small.tile([P, 1], f32)
            nc.vector.memset(pre_in, 0.0)
            pre_out = small.tile([P, 1], f32)
            nc.scalar.activation(
                pre_out, pre_in, mybir.ActivationFunctionType.Sqrt, bias=1.0
            )

            ident_t = singles.tile([P, P], agg_dt)   # transpose rhs + self-loop
            make_identity(nc, ident_t)

            # x tiles: x_t[p, f] = x[t*P + p, f]; issued upfront on the
            # otherwise-idle sync HWDGE queue (adj rides the SWDGE queue)
            x_tiles = [
                x_pool.tile([P, F], f32, name=f"x_t{t}", tag=f"x{t}")
                for t in range(NT)
            ]
            for t in range(NT):
                nc.sync.dma_start(x_tiles[t], x_d[P * t : P * (t + 1), :])

            # W first on the scalar HWDGE ring (the b broadcast below is a
            # slow 128-descriptor replication spray; W must not sit behind
            # it -- the W path feeds the linear tail)
            w_sb = singles.tile([P, P], f32)
            nc.scalar.dma_start(w_sb, w_d[:, :])
            # bias broadcast across partitions: b_sb[p, o] = b[o]
            b_sb = singles.tile([P, F], f32)
            b_ap = b_d[:]
            nc.scalar.dma_start(
                b_sb, bass.AP(tensor=b_ap.tensor, offset=b_ap.offset, ap=[[0, P], *b_ap.ap])
            )

            # regular (HAM-visible) matmul activity during the DMA fill
            # phase: identity x identity into scratch PSUM (fp32 out, so
            # [P,4,P] f32 matches the 2KiB bank footprint of the fp16
            # transpose tiles sharing this pool slot)
            for w in range(warm // 4):
                wtp = tp_psum.tile([P, 4, P], f32, name="tp", tag="tp")
                for s in range(4):
                    nc.tensor.matmul(
                        wtp[:, s, :], ident_t, ident_t, start=True, stop=True
                    )

            # W cast fp32->fp16 on DVE (idle early); the transpose itself is
            # deferred to the flush so the in-order PE queue never stalls on
            # the W DMA ahead of the adj tile transposes
            w16 = singles.tile([P, P], agg_dt)
            nc.vector.tensor_copy(w16, w_sb)
            wt_sb = singles.tile([P, P], agg_dt)

            junk_sb = singles.tile([P, N], agg_dt)     # reduce pass scratch out
            xp_sb = singles.tile([P, NT, F], agg_dt)   # x' = d * x
            at_sb = singles.tile([P, NT, N], agg_dt)   # resident A^T
            aggt_sb = singles.tile([P, N], agg_dt)     # aggT = (A+I)x' transposed
            out_sb = singles.tile([P, NT, F], f32)
            d_all = singles.tile([P, NT], f32)         # d = (rowsum+1)^-1/2

            # one slot per accumulator: distinct tags keep all NCH tiles
            # simultaneously resident (they accumulate across the whole kernel)
            agg_ps = [
                agg_psum.tile([P, NCHUNK], f32, name=f"agg_ps{i}", tag=f"agg{i}")
                for i in range(NCH)
            ]

            # aggregation ops per chunk: 16 regular j-steps + TPC identity
            # (self-loop) steps; each ready at a known row-tile time
            agg_plan = [[] for _ in range(NCH)]
            for i in range(NCH):
                ready_i = TPC * i + TPC - 1  # chunk's AT columns complete
                for j in range(NT):
                    agg_plan[i].append((max(ready_i, j), "j", j))
                for u in range(TPC * i, TPC * (i + 1)):
                    agg_plan[i].append((max(ready_i, u), "ident", u))
            agg_emitted = [0] * NCH
            agg_pending = []  # ready steps carried across tiles (burst cap)

            def emit_agg(i, kind, idx):
                total = len(agg_plan[i])
                first = agg_emitted[i] == 0
                agg_emitted[i] += 1
                last = agg_emitted[i] == total
                if kind == "j":
                    rhs = at_sb[:, idx, NCHUNK * i : NCHUNK * (i + 1)]
                    out_ap = agg_ps[i]
                else:
                    # self-loop: aggT[:, u] += x'_u^T (= x'_u^T @ I)
                    rhs = ident_t
                    off = P * (idx - TPC * i)
                    out_ap = agg_ps[i][:, off : off + P]
                nc.tensor.matmul(
                    out_ap,
                    xp_sb[:, idx, :],
                    rhs,
                    start=first,
                    stop=last,
                )

            def fire_agg_steps(t, phase, budget):
                # the last tile's ready steps are appended (and ordered)
                # by the flush below, but its bursts still fire backlog
                # so those matmuls overlap the final transposes
                if phase == 0 and t < NT - 1:
                    for i in range(NCH):
                        for ready, kind, idx in agg_plan[i]:
                            if ready == t:
                                agg_pending.append((i, kind, idx))
                # steps touching xp_t never fire at tile t (xp_t lands
                # ~2.4us after the tile's DMA; a premature matmul would
                # stall the in-order PE queue ahead of the next tile's
                # transposes) — they carry over in the pending list
                fired = 0
                k = 0
                while fired < budget and k < len(agg_pending):
                    i, kind, idx = agg_pending[k]
                    if idx >= t:
                        k += 1
                        continue
                    agg_pending.pop(k)
                    emit_agg(i, kind, idx)
                    fired += 1

            for t in range(NT):
                if t + PREF < NT:
                    load_adj(t + PREF)
                a_c = a_tiles.pop(t)

                # rowsum: one full-width ACT pass with accumulator (the
                # wide copy output is discarded scratch; every accum path
                # runs ~1 el/cyc, and ACT's is the cheapest at 1.2 GHz),
                # then sqrt on the SAME engine — no cross-engine hop.
                # The LAST tile splits the pass DVE/ACT to shorten the
                # tail-entry degree chain (DVE is idle at that point).
                rs = small.tile([P, 1], f32, name=f"rs{t}", tag=f"rs{t}")
                if t in SPLIT:
                    rs_a = small.tile([P, 1], f32, name=f"rsh{t}", tag=f"rsh{t}")
                    nc.scalar.activation(
                        junk_sb[:, : N // 2],
                        a_c[:, : N // 2],
                        mybir.ActivationFunctionType.Copy,
                        accum_out=rs_a,
                    )
                    rs_b = small.tile([P, 1], f32, name=f"rsi{t}", tag=f"rsi{t}")
                    nc.scalar.activation(
                        junk_sb[:, N // 2 :],
                        a_c[:, N // 2 :],
                        mybir.ActivationFunctionType.Copy,
                        accum_out=rs_b,
                    )
                    nc.gpsimd.tensor_add(rs, rs_a, rs_b)
                elif t < NT - 1:
                    nc.scalar.activation(
                        junk_sb,
                        a_c,
                        mybir.ActivationFunctionType.Copy,
                        accum_out=rs,
                    )
                else:
                    # last tile: the whole reduce runs on DVE at high
                    # priority so it starts the moment the tile lands,
                    # ahead of queued transpose drains; ACT only does the
                    # sqrt, so its queue never blocks the d-chain
                    with tc.high_priority():
                        nc.vector.tensor_scalar(
                            junk_sb,
                            a_c,
                            1.0,
                            None,
                            op0=mybir.AluOpType.mult,
                            op1=mybir.AluOpType.add,  # accum reduce op
                            accum_out=rs,
                        )
                # the d-chain gates the aggregation matmuls; mark it
                # high-priority so the scheduler never queues a big
                # late-input op (e.g. the next tile's rowsum pass) ahead of
                # these tiny ops in the in-order engine queues
                sq = small.tile([P, 1], f32, name=f"sq{t}", tag=f"sq{t}")
                with tc.high_priority():
                    nc.scalar.activation(
                        sq, rs, mybir.ActivationFunctionType.Sqrt, bias=1.0
                    )
                    # d-chain tail on DVE: d, then x' = d * x
                    nc.vector.reciprocal(d_all[:, t : t + 1], sq)
                    nc.vector.tensor_scalar_mul(
                        xp_sb[:, t, :], x_tiles[t], d_all[:, t : t + 1]
                    )

                # transpose all 16 blocks on the PE in 2 groups of 8
                # (one PSUM bank each); both drains on DVE, whose plain
                # PSUM copy runs in 2x mode (~0.6 ns/el vs ACT's 1x)
                for gi, g0 in enumerate(range(0, NT, 8)):
                    tp = tp_psum.tile([P, 8, P], agg_dt, name="tp", tag="tp")
                    for s in range(8):
                        j = g0 + s
                        nc.tensor.transpose(
                            tp[:, s, :], a_c[:, P * j : P * (j + 1)], ident_t
                        )
                    dst = at_sb[:, g0 : g0 + 8, P * t : P * (t + 1)]
                    if t == NT - 1:
                        # last tile: both drains on ACT so DVE stays free
                        # for the reduce + d-chain that gate the flush
                        nc.scalar.copy(dst, tp)
                    else:
                        nc.vector.tensor_copy(dst, tp)
                    budget = cap // 2 + (gi and cap % 2)
                    if t >= NT - 2:
                        # keep the PE queue clear right before/at the last
                        # tile so its transposes run the moment it lands;
                        # the backlog drains in the flush instead
                        budget = 1
                    fire_agg_steps(t, gi, budget)

            # W^T on the PE now that all tile transposes are queued: wt
            # never gates the adj stream, only the linear tail
            wt_ps = tp_psum.tile([P, 8, P], agg_dt, name="tp", tag="tp")
            nc.tensor.transpose(wt_ps[:, 0, :], w16, ident_t)
            nc.vector.tensor_copy(wt_sb, wt_ps[:, 0, :])

            # final flush: close chunks 0..NCH-2 first (their remaining
            # steps are ready before the last chunk's AT drains land) so
            # their drain/linear/write chains overlap chunk NCH-1's burst
            t = NT - 1
            for i in range(NCH):
                for ready, kind, idx in agg_plan[i]:
                    if ready == t:
                        agg_pending.append((i, kind, idx))
            agg_pending.sort(key=lambda s: (s[0] == NCH - 1, s[0]))

            def drain_chunk(i):
                dst = aggt_sb[:, NCHUNK * i : NCHUNK * (i + 1)]
                if i % 2 == 0:
                    nc.vector.tensor_copy(dst, agg_ps[i])
                else:
                    nc.scalar.copy(dst, agg_ps[i])

            def emit_group(i):
                # linear layer for chunk i's 4 row-tiles: 4 matmuls into
                # a single PSUM bank (no slot-recycle pressure), then
                # out = d[n]*o2 + b per tile (d is per-PARTITION in o2's
                # [n, o] layout), and one 4-tile store
                # the linear output reuses chunk i's (drained) agg bank, so
                # the four groups have independent PSUM and never serialize
                # on slot recycling
                o2 = agg_psum.tile([P, NCHUNK], f32, name=f"o2_{i}", tag=f"agg{i}")
                for k in range(TPC):
                    u = TPC * i + k
                    nc.tensor.matmul(
                        o2[:, P * k : P * (k + 1)],
                        aggt_sb[:, P * u : P * (u + 1)],
                        wt_sb,
                        start=True,
                        stop=True,
                    )
                for k in range(TPC):
                    u = TPC * i + k
                    nc.vector.scalar_tensor_tensor(
                        out_sb[:, u, :],
                        o2[:, P * k : P * (k + 1)],
                        d_all[:, u : u + 1],
                        b_sb,
                        op0=mybir.AluOpType.mult,
                        op1=mybir.AluOpType.add,
                    )
                u0 = TPC * i
                dst = bass.AP(
                    tensor=o_d[:, :].tensor,
                    offset=u0 * P * F,
                    ap=[[F, P], [P * F, TPC], [1, F]],
                )
                eng = nc.sync if i % 2 == 0 else nc.scalar
                eng.dma_start(dst, out_sb[:, u0 : u0 + TPC, :])

            # flush order: (1) leftover xp-ready steps of chunks 0..2,
            # (2) their j=15 closes (gated only on xp_15, which lands
            # before tile 15's AT drains) + drains, (3) the last chunk's
            # burst, so chunks 0..2's linear/store chains overlap it
            backlog = [
                s for s in agg_pending if s[0] != NCH - 1 and s[2] != NT - 1
            ]
            close_e = [
                s for s in agg_pending if s[0] != NCH - 1 and s[2] == NT - 1
            ]
            c3_bulk = [
                s for s in agg_pending if s[0] == NCH - 1 and s[2] != NT - 1
            ]
            c3_last = [
                s for s in agg_pending if s[0] == NCH - 1 and s[2] == NT - 1
            ]
            for s in backlog:
                emit_agg(*s)
            for s in close_e:
                emit_agg(*s)
            for i in range(NCH - 1):
                drain_chunk(i)
            for s in c3_bulk:
                emit_agg(*s)
            for s in c3_last:
                emit_agg(*s)
            for i in range(NCH - 1):
                emit_group(i)
            drain_chunk(NCH - 1)
            emit_group(NCH - 1)

    nc.compile()
    return nc


def get_program(agg_dtype_name="float16", bufs_a=10, warm=WARMUP, cap=AGG_CAP):
    key = (agg_dtype_name, bufs_a, warm, cap)
    if key not in _PROGRAM_CACHE:
        _PROGRAM_CACHE[key] = _build_program(agg_dtype_name, bufs_a, warm, cap)
    return _PROGRAM_CACHE[key]


def kernel(x, adj, W, b, _trace=False, _agg_dtype="float16", _pe_blocks=None,
           _trace_cores=None, _bufs_a=10, _warm=WARMUP, _cap=AGG_CAP):
    from concourse.bass_utils import run_bass_kernel_spmd

    if _trace_cores is None:
        _trace_cores = [0]
    nc = get_program(_agg_dtype, _bufs_a, _warm, _cap)
    x = np.ascontiguousarray(np.asarray(x), dtype=np.float32)
    adj = np.ascontiguousarray(np.asarray(adj), dtype=np.float32)
    W = np.ascontiguousarray(np.asarray(W), dtype=np.float32)
    b = np.ascontiguousarray(np.asarray(b), dtype=np.float32)

    in_maps = [
        {"x": x[c], "adj": adj[c], "W": W, "b": b} for c in range(B)
    ]
    res = run_bass_kernel_spmd(
        nc, in_maps, list(range(B)), trace=_trace,
        trace_cores=_trace_cores if _trace else None,
    )
    out = np.stack([res.results[c]["out"] for c in range(B)], axis=0)
    if _trace:
        return out, res
    return out



# revision 39
# speedup vs baseline: 1.0049x; 1.0049x over previous
"""GCN layer kernel for Trainium2, batch-parallel across 8 NeuronCores.

out[b] = D^-1/2 (A[b]+I) D^-1/2 @ x[b] @ W.T + b_vec

Per-core dataflow (core c owns batch element c):
  - adj row-tiles [128, N] are loaded HBM->SBUF with an fp32->fp16 cast
    done inline by the SWDGE DMA path (gpsimd queue), so no engine pass
    is spent on casting and no fp32 staging buffer is needed.
  - Degrees: one DVE tensor_reduce per row-tile over the fp16 tile
    (fp32 accumulate), then d = Rsqrt(rowsum + 1) on the scalar engine.
  - The PE transposes each 128x128 block into a resident A^T buffer in
    SBUF (matmul needs the contraction index on the partition dim).
    Transpose-PSUM drains alternate between the scalar and vector
    engines so neither becomes the bottleneck.
  - The degree scaling is folded into x (x' = d*x) and the output
    (out = d * (...)), so adj_norm is never materialized.
  - Aggregation runs as PSUM-accumulated matmuls aggT[f, n] += x'_j^T AT_j,
    fired as soon as their row-tile dependencies are satisfied. The +I
    self-loop is folded in as x'_u^T @ I identity matmuls on the PE.
  - Warmup: a burst of regular (non-transpose) identity matmuls during
    the DMA fill phase lifts the HAM clock gate to K=8/8 early.
  - Tail: chunks 0..2 are closed (their j=15 steps fired) before chunk
    3's big burst so their drain/linear/write chains overlap it.
  - Tail: out[n,o] = d[n] * (aggT^T @ W^T)[n,o] + b[o], written out in
    2-tile batches alternating across both HWDGE queues.
"""

import numpy as np

B, N, F = 8, 2048, 128
P = 128                # partition tile / block size
NT = N // P            # 16 row tiles
NCHUNK = 512           # moving-dim chunk for the aggregation matmul
NCH = N // NCHUNK      # 4 chunks
TPC = NCHUNK // P      # row tiles per chunk
WARMUP = 32            # regular matmuls at start to lift the HAM clock gate
AGG_CAP = 12           # aggregation matmuls fired per row-tile

_PROGRAM_CACHE = {}


def _build_program(agg_dtype_name="float16", bufs_a=10, warm=WARMUP, cap=AGG_CAP):
    import concourse.bacc as bacc
    import concourse.bass as bass
    import concourse.tile as tile
    from concourse import mybir
    from concourse.masks import make_identity

    f32 = mybir.dt.float32
    agg_dt = getattr(mybir.dt, agg_dtype_name)

    nc = bacc.Bacc(
        "TRN2",
        target_bir_lowering=False,
        debug=False,
        num_devices=B,
        # no cross-core branching: per-core data arrives via in_maps, so
        # skip the partition-id register load sequence in the preamble
        enable_partition_id=False,
    )
    x_d = nc.dram_tensor("x", [N, F], f32, kind="ExternalInput")
    a_d = nc.dram_tensor("adj", [N, N], f32, kind="ExternalInput")
    w_d = nc.dram_tensor("W", [F, F], f32, kind="ExternalInput")
    b_d = nc.dram_tensor("b", [F], f32, kind="ExternalInput")
    o_d = nc.dram_tensor("out", [N, F], f32, kind="ExternalOutput")

    with tile.TileContext(nc) as tc:
        with (
            tc.tile_pool(name="singles", bufs=1) as singles,
            tc.tile_pool(name="a_cast", bufs=bufs_a) as a_cast,
            tc.tile_pool(name="x_pool", bufs=1) as x_pool,
            tc.tile_pool(name="small", bufs=6) as small,
            tc.tile_pool(name="tp_psum", bufs=4, space="PSUM") as tp_psum,
            tc.tile_pool(name="agg_psum", bufs=1, space="PSUM") as agg_psum,
        ):
            # prefetch: the first adj tiles are dispatched before ANY
            # other gpsimd work (make_identity runs on gpsimd and would
            # otherwise delay the first SWDGE load by ~1.5us); later
            # tiles are issued from inside the loop (slot-gated). The
            # adj stream is engine-paced, not DMA-bound, so small 1 MiB
            # tiles win: the first tile lands ~3us sooner than a 2 MiB
            # chunk would, and the whole pipeline shifts with it.
            PREF = 4
            SPLIT = {0, 1}   # tiles loaded as two half-DMAs (earlier start)
            a_tiles = {}

            def load_adj(t):
                # fp32 HBM -> fp16 SBUF, cast inline on the SWDGE path.
                # The first tiles are split into two half-row DMAs so the
                # degree pass and the first transposes start ~4us sooner.
                a_c = a_cast.tile([P, N], agg_dt, name="a_c")
                if t in SPLIT:
                    h = N // 2
                    nc.gpsimd.dma_start(a_c[:, :h], a_d[P * t : P * (t + 1), :h])
                    nc.gpsimd.dma_start(a_c[:, h:], a_d[P * t : P * (t + 1), h:])
                else:
                    nc.gpsimd.dma_start(a_c, a_d[P * t : P * (t + 1), :])
                a_tiles[t] = a_c

            for t in range(PREF):
                load_adj(t)

            # preload the Sqrt activation table immediately (no DMA deps:
            # the input is a DVE memset) -- avoids a ~2.7us ACT_TABLE_LOAD
            # stall at the first degree compute
            pre_in = small.tile([P, 1], f32)
            nc.vector.memset(pre_in, 0.0)
            pre_out = small.tile([P, 1], f32)
            nc.scalar.activation(
                pre_out, pre_in, mybir.ActivationFunctionType.Sqrt, bias=1.0
            )

            ident_t = singles.tile([P, P], agg_dt)   # transpose rhs + self-loop
            make_identity(nc, ident_t)

            # x tiles: x_t[p, f] = x[t*P + p, f]; issued upfront on the
            # otherwise-idle sync HWDGE queue (adj rides the SWDGE queue)
            x_tiles = [
                x_pool.tile([P, F], f32, name=f"x_t{t}", tag=f"x{t}")
                for t in range(NT)
            ]
            for t in range(NT):
                nc.sync.dma_start(x_tiles[t], x_d[P * t : P * (t + 1), :])

            # W first on the scalar HWDGE ring (the b broadcast below is a
            # slow 128-descriptor replication spray; W must not sit behind
            # it -- the W path feeds the linear tail)
            w_sb = singles.tile([P, P], f32)
            nc.scalar.dma_start(w_sb, w_d[:, :])
            # bias broadcast across partitions: b_sb[p, o] = b[o]
            b_sb = singles.tile([P, F], f32)
            b_ap = b_d[:]
            nc.scalar.dma_start(
                b_sb, bass.AP(tensor=b_ap.tensor, offset=b_ap.offset, ap=[[0, P], *b_ap.ap])
            )

            # regular (HAM-visible) matmul activity during the DMA fill
            # phase: identity x identity into scratch PSUM (fp32 out, so
            # [P,4,P] f32 matches the 2KiB bank footprint of the fp16
            # transpose tiles sharing this pool slot)
            for w in range(warm // 4):
                wtp = tp_psum.tile([P, 4, P], f32, name="tp", tag="tp")
                for s in range(4):
                    nc.tensor.matmul(
                        wtp[:, s, :], ident_t, ident_t, start=True, stop=True
                    )

            # W cast fp32->fp16 on DVE (idle early); the transpose itself is
            # deferred to the flush so the in-order PE queue never stalls on
            # the W DMA ahead of the adj tile transposes
            w16 = singles.tile([P, P], agg_dt)
            nc.vector.tensor_copy(w16, w_sb)
            wt_sb = singles.tile([P, P], agg_dt)

            junk_sb = singles.tile([P, N], agg_dt)     # reduce pass scratch out
            xp_sb = singles.tile([P, NT, F], agg_dt)   # x' = d * x
            at_sb = singles.tile([P, NT, N], agg_dt)   # resident A^T
            aggt_sb = singles.tile([P, N], agg_dt)     # aggT = (A+I)x' transposed
            out_sb = singles.tile([P, NT, F], f32)
            d_all = singles.tile([P, NT], f32)         # d = (rowsum+1)^-1/2

            # one slot per accumulator: distinct tags keep all NCH tiles
            # simultaneously resident (they accumulate across the whole kernel)
            agg_ps = [
                agg_psum.tile([P, NCHUNK], f32, name=f"agg_ps{i}", tag=f"agg{i}")
                for i in range(NCH)
            ]

            # aggregation ops per chunk: 16 regular j-steps + TPC identity
            # (self-loop) steps; each ready at a known row-tile time
            agg_plan = [[] for _ in range(NCH)]
            for i in range(NCH):
                ready_i = TPC * i + TPC - 1  # chunk's AT columns complete
                for j in range(NT):
                    agg_plan[i].append((max(ready_i, j), "j", j))
                for u in range(TPC * i, TPC * (i + 1)):
                    agg_plan[i].append((max(ready_i, u), "ident", u))
            agg_emitted = [0] * NCH
            agg_pending = []  # ready steps carried across tiles (burst cap)

            def emit_agg(i, kind, idx):
                total = len(agg_plan[i])
                first = agg_emitted[i] == 0
                agg_emitted[i] += 1
                last = agg_emitted[i] == total
                if kind == "j":
                    rhs = at_sb[:, idx, NCHUNK * i : NCHUNK * (i + 1)]
                    out_ap = agg_ps[i]
                else:
                    # self-loop: aggT[:, u] += x'_u^T (= x'_u^T @ I)
                    rhs = ident_t
                    off = P * (idx - TPC * i)
                    out_ap = agg_ps[i][:, off : off + P]
                nc.tensor.matmul(
                    out_ap,
                    xp_sb[:, idx, :],
                    rhs,
                    start=first,
                    stop=last,
                )

            def fire_agg_steps(t, phase, budget):
                # the last tile's ready steps are appended (and ordered)
                # by the flush below, but its bursts still fire backlog
                # so those matmuls overlap the final transposes
                if phase == 0 and t < NT - 1:
                    for i in range(NCH):
                        for ready, kind, idx in agg_plan[i]:
                            if ready == t:
                                agg_pending.append((i, kind, idx))
                # steps touching xp_t never fire at tile t (xp_t lands
                # ~2.4us after the tile's DMA; a premature matmul would
                # stall the in-order PE queue ahead of the next tile's
                # transposes) — they carry over in the pending list
                fired = 0
                k = 0
                while fired < budget and k < len(agg_pending):
                    i, kind, idx = agg_pending[k]
                    if idx >= t:
                        k += 1
                        continue
                    agg_pending.pop(k)
                    emit_agg(i, kind, idx)
                    fired += 1

            for t in range(NT):
                if t + PREF < NT:
                    load_adj(t + PREF)
                a_c = a_tiles.pop(t)

                # rowsum: one full-width ACT pass with accumulator (the
                # wide copy output is discarded scratch; every accum path
                # runs ~1 el/cyc, and ACT's is the cheapest at 1.2 GHz),
                # then sqrt on the SAME engine — no cross-engine hop.
                # The LAST tile splits the pass DVE/ACT to shorten the
                # tail-entry degree chain (DVE is idle at that point).
                rs = small.tile([P, 1], f32, name=f"rs{t}", tag=f"rs{t}")
                if t in SPLIT:
                    rs_a = small.tile([P, 1], f32, name=f"rsh{t}", tag=f"rsh{t}")
                    nc.scalar.activation(
                        junk_sb[:, : N // 2],
                        a_c[:, : N // 2],
                        mybir.ActivationFunctionType.Copy,
                        accum_out=rs_a,
                    )
                    rs_b = small.tile([P, 1], f32, name=f"rsi{t}", tag=f"rsi{t}")
                    nc.scalar.activation(
                        junk_sb[:, N // 2 :],
                        a_c[:, N // 2 :],
                        mybir.ActivationFunctionType.Copy,
                        accum_out=rs_b,
                    )
                    nc.gpsimd.tensor_add(rs, rs_a, rs_b)
                elif t < NT - 1:
                    nc.scalar.activation(
                        junk_sb,
                        a_c,
                        mybir.ActivationFunctionType.Copy,
                        accum_out=rs,
                    )
                else:
                    # last tile: the whole reduce runs on DVE at high
                    # priority so it starts the moment the tile lands,
                    # ahead of queued transpose drains; ACT only does the
                    # sqrt, so its queue never blocks the d-chain
                    with tc.high_priority():
                        nc.vector.tensor_scalar(
                            junk_sb,
                            a_c,
                            1.0,
                            None,
                            op0=mybir.AluOpType.mult,
                            op1=mybir.AluOpType.add,  # accum reduce op
                            accum_out=rs,
                        )
                # the d-chain gates the aggregation matmuls; mark it
                # high-priority so the scheduler never queues a big
                # late-input op (e.g. the next tile's rowsum pass) ahead of
                # these tiny ops in the in-order engine queues
                sq = small.tile([P, 1], f32, name=f"sq{t}", tag=f"sq{t}")
                with tc.high_priority():
                    nc.scalar.activation(
                        sq, rs, mybir.ActivationFunctionType.Sqrt, bias=1.0
                    )
                    # d-chain tail on DVE: d, then x' = d * x
                    nc.vector.reciprocal(d_all[:, t : t + 1], sq)
                    nc.vector.tensor_scalar_mul(
                        xp_sb[:, t, :], x_tiles[t], d_all[:, t : t + 1]
                    )

                # transpose all 16 blocks on the PE in 2 groups of 8
                # (one PSUM bank each); both drains on DVE, whose plain
                # PSUM copy runs in 2x mode (~0.6 ns/el vs ACT's 1x)
                for gi, g0 in enumerate(range(0, NT, 8)):
                    tp = tp_psum.tile([P, 8, P], agg_dt, name="tp", tag="tp")
                    for s in range(8):
                        j = g0 + s
                        nc.tensor.transpose(
                            tp[:, s, :], a_c[:, P * j : P * (j + 1)], ident_t
                        )
                    dst = at_sb[:, g0 : g0 + 8, P * t : P * (t + 1)]
                    if t == NT - 1:
                        # last tile: both drains on ACT so DVE stays free
                        # for the reduce + d-chain that gate the flush
                        nc.scalar.copy(dst, tp)
                    else:
                        nc.vector.tensor_copy(dst, tp)
                    fire_agg_steps(t, gi, cap // 2 + (gi and cap % 2))

            # W^T on the PE now that all tile transposes are queued: wt
            # never gates the adj stream, only the linear tail
            wt_ps = tp_psum.tile([P, 8, P], agg_dt, name="tp", tag="tp")
            nc.tensor.transpose(wt_ps[:, 0, :], w16, ident_t)
            nc.vector.tensor_copy(wt_sb, wt_ps[:, 0, :])

            # final flush: close chunks 0..NCH-2 first (their remaining
            # steps are ready before the last chunk's AT drains land) so
            # their drain/linear/write chains overlap chunk NCH-1's burst
            t = NT - 1
            for i in range(NCH):
                for ready, kind, idx in agg_plan[i]:
                    if ready == t:
                        agg_pending.append((i, kind, idx))
            agg_pending.sort(key=lambda s: (s[0] == NCH - 1, s[0]))

            def drain_chunk(i):
                dst = aggt_sb[:, NCHUNK * i : NCHUNK * (i + 1)]
                if i % 2 == 0:
                    nc.vector.tensor_copy(dst, agg_ps[i])
                else:
                    nc.scalar.copy(dst, agg_ps[i])

            def emit_group(i):
                # linear layer for chunk i's 4 row-tiles: 4 matmuls into
                # a single PSUM bank (no slot-recycle pressure), then
                # out = d[n]*o2 + b per tile (d is per-PARTITION in o2's
                # [n, o] layout), and one 4-tile store
                # the linear output reuses chunk i's (drained) agg bank, so
                # the four groups have independent PSUM and never serialize
                # on slot recycling
                o2 = agg_psum.tile([P, NCHUNK], f32, name=f"o2_{i}", tag=f"agg{i}")
                for k in range(TPC):
                    u = TPC * i + k
                    nc.tensor.matmul(
                        o2[:, P * k : P * (k + 1)],
                        aggt_sb[:, P * u : P * (u + 1)],
                        wt_sb,
                        start=True,
                        stop=True,
                    )
                for k in range(TPC):
                    u = TPC * i + k
                    nc.vector.scalar_tensor_tensor(
                        out_sb[:, u, :],
                        o2[:, P * k : P * (k + 1)],
                        d_all[:, u : u + 1],
                        b_sb,
                        op0=mybir.AluOpType.mult,
                        op1=mybir.AluOpType.add,
                    )
                u0 = TPC * i
                dst = bass.AP(
                    tensor=o_d[:, :].tensor,
                    offset=u0 * P * F,
                    ap=[[F, P], [P * F, TPC], [1, F]],
                )
                eng = nc.sync if i % 2 == 0 else nc.scalar
                eng.dma_start(dst, out_sb[:, u0 : u0 + TPC, :])

            # flush order: (1) leftover xp-ready steps of chunks 0..2,
            # (2) their j=15 closes (gated only on xp_15, which lands
            # before tile 15's AT drains) + drains, (3) the last chunk's
            # burst, so chunks 0..2's linear/store chains overlap it
            ready_now = [s for s in agg_pending if s[2] != NT - 1]
            need_last = [s for s in agg_pending if s[2] == NT - 1]
            ready_now.sort(key=lambda s: s[0] == NCH - 1)
            need_last.sort(key=lambda s: s[0] == NCH - 1)
            for s in ready_now:
                emit_agg(*s)
            for s in need_last:
                emit_agg(*s)
            for i in range(NCH - 1):
                drain_chunk(i)
            for i in range(NCH - 1):
                emit_group(i)
            drain_chunk(NCH - 1)
            emit_group(NCH - 1)

    nc.compile()
    return nc


def get_program(agg_dtype_name="float16", bufs_a=10, warm=WARMUP, cap=AGG_CAP):
    key = (agg_dtype_name, bufs_a, warm, cap)
    if key not in _PROGRAM_CACHE:
        _PROGRAM_CACHE[key] = _build_program(agg_dtype_name, bufs_a, warm, cap)
    return _PROGRAM_CACHE[key]


def kernel(x, adj, W, b, _trace=False, _agg_dtype="float16", _pe_blocks=None,
           _trace_cores=None, _bufs_a=10, _warm=WARMUP, _cap=AGG_CAP):
    from concourse.bass_utils import run_bass_kernel_spmd

    if _trace_cores is None:
        _trace_cores = [0]
    nc = get_program(_agg_dtype, _bufs_a, _warm, _cap)
    x = np.ascontiguousarray(np.asarray(x), dtype=np.float32)
    adj = np.ascontiguousarray(np.asarray(adj), dtype=np.float32)
    W = np.ascontiguousarray(np.asarray(W), dtype=np.float32)
    b = np.ascontiguousarray(np.asarray(b), dtype=np.float32)

    in_maps = [
        {"x": x[c], "adj": adj[c], "W": W, "b": b} for c in range(B)
    ]
    res = run_bass_kernel_spmd(
        nc, in_maps, list(range(B)), trace=_trace,
        trace_cores=_trace_cores if _trace else None,
    )
    out = np.stack([res.results[c]["out"] for c in range(B)], axis=0)
    if _trace:
        return out, res
    return out



# revision 40
# speedup vs baseline: 1.0500x; 1.0449x over previous
"""GCN layer kernel for Trainium2, batch-parallel across 8 NeuronCores.

out[b] = D^-1/2 (A[b]+I) D^-1/2 @ x[b] @ W.T + b_vec

Per-core dataflow (core c owns batch element c):
  - adj row-tiles [128, N] are loaded HBM->SBUF with an fp32->fp16 cast
    done inline by the SWDGE DMA path (gpsimd queue), so no engine pass
    is spent on casting and no fp32 staging buffer is needed.
  - Degrees: one DVE tensor_reduce per row-tile over the fp16 tile
    (fp32 accumulate), then d = Rsqrt(rowsum + 1) on the scalar engine.
  - The PE transposes each 128x128 block into a resident A^T buffer in
    SBUF (matmul needs the contraction index on the partition dim).
    Transpose-PSUM drains alternate between the scalar and vector
    engines so neither becomes the bottleneck.
  - The degree scaling is folded into x (x' = d*x) and the output
    (out = d * (...)), so adj_norm is never materialized.
  - Aggregation runs as PSUM-accumulated matmuls aggT[f, n] += x'_j^T AT_j,
    fired as soon as their row-tile dependencies are satisfied. The +I
    self-loop is folded in as x'_u^T @ I identity matmuls on the PE.
  - Warmup: a burst of regular (non-transpose) identity matmuls during
    the DMA fill phase lifts the HAM clock gate to K=8/8 early.
  - Tail: chunks 0..2 are closed (their j=15 steps fired) before chunk
    3's big burst so their drain/linear/write chains overlap it.
  - Tail: out[n,o] = d[n] * (aggT^T @ W^T)[n,o] + b[o], written out in
    2-tile batches alternating across both HWDGE queues.
"""

import numpy as np

B, N, F = 8, 2048, 128
P = 128                # partition tile / block size
NT = N // P            # 16 row tiles
NCHUNK = 512           # moving-dim chunk for the aggregation matmul
NCH = N // NCHUNK      # 4 chunks
TPC = NCHUNK // P      # row tiles per chunk
WARMUP = 32            # regular matmuls at start to lift the HAM clock gate
AGG_CAP = 12           # aggregation matmuls fired per row-tile

_PROGRAM_CACHE = {}


def _build_program(agg_dtype_name="float16", bufs_a=10, warm=WARMUP, cap=AGG_CAP):
    import concourse.bacc as bacc
    import concourse.bass as bass
    import concourse.tile as tile
    from concourse import mybir
    from concourse.masks import make_identity

    f32 = mybir.dt.float32
    agg_dt = getattr(mybir.dt, agg_dtype_name)

    nc = bacc.Bacc(
        "TRN2",
        target_bir_lowering=False,
        debug=False,
        num_devices=B,
        # no cross-core branching: per-core data arrives via in_maps, so
        # skip the partition-id register load sequence in the preamble
        enable_partition_id=False,
    )
    x_d = nc.dram_tensor("x", [N, F], f32, kind="ExternalInput")
    a_d = nc.dram_tensor("adj", [N, N], f32, kind="ExternalInput")
    w_d = nc.dram_tensor("W", [F, F], f32, kind="ExternalInput")
    b_d = nc.dram_tensor("b", [F], f32, kind="ExternalInput")
    o_d = nc.dram_tensor("out", [N, F], f32, kind="ExternalOutput")

    with tile.TileContext(nc) as tc:
        with (
            tc.tile_pool(name="singles", bufs=1) as singles,
            tc.tile_pool(name="a_cast", bufs=bufs_a) as a_cast,
            tc.tile_pool(name="x_pool", bufs=1) as x_pool,
            tc.tile_pool(name="small", bufs=6) as small,
            tc.tile_pool(name="tp_psum", bufs=4, space="PSUM") as tp_psum,
            tc.tile_pool(name="agg_psum", bufs=1, space="PSUM") as agg_psum,
        ):
            # prefetch: the first adj tiles are dispatched before ANY
            # other gpsimd work (make_identity runs on gpsimd and would
            # otherwise delay the first SWDGE load by ~1.5us); later
            # tiles are issued from inside the loop (slot-gated). The
            # adj stream is engine-paced, not DMA-bound, so small 1 MiB
            # tiles win: the first tile lands ~3us sooner than a 2 MiB
            # chunk would, and the whole pipeline shifts with it.
            PREF = 4
            SPLIT = {0, 1}   # tiles loaded as two half-DMAs (earlier start)
            a_tiles = {}

            def load_adj(t):
                # fp32 HBM -> fp16 SBUF, cast inline on the SWDGE path.
                # The first tiles are split into two half-row DMAs so the
                # degree pass and the first transposes start ~4us sooner.
                a_c = a_cast.tile([P, N], agg_dt, name="a_c")
                if t in SPLIT:
                    h = N // 2
                    nc.gpsimd.dma_start(a_c[:, :h], a_d[P * t : P * (t + 1), :h])
                    nc.gpsimd.dma_start(a_c[:, h:], a_d[P * t : P * (t + 1), h:])
                else:
                    nc.gpsimd.dma_start(a_c, a_d[P * t : P * (t + 1), :])
                a_tiles[t] = a_c

            for t in range(PREF):
                load_adj(t)

            # preload the Sqrt activation table immediately (no DMA deps:
            # the input is a DVE memset) -- avoids a ~2.7us ACT_TABLE_LOAD
            # stall at the first degree compute
            pre_in = small.tile([P, 1], f32)
            nc.vector.memset(pre_in, 0.0)
            pre_out = small.tile([P, 1], f32)
            nc.scalar.activation(
                pre_out, pre_in, mybir.ActivationFunctionType.Sqrt, bias=1.0
            )

            ident_t = singles.tile([P, P], agg_dt)   # transpose rhs + self-loop
            make_identity(nc, ident_t)

            # x tiles: x_t[p, f] = x[t*P + p, f]; issued upfront on the
            # otherwise-idle sync HWDGE queue (adj rides the SWDGE queue)
            x_tiles = [
                x_pool.tile([P, F], f32, name=f"x_t{t}", tag=f"x{t}")
                for t in range(NT)
            ]
            for t in range(NT):
                nc.sync.dma_start(x_tiles[t], x_d[P * t : P * (t + 1), :])

            # W first on the scalar HWDGE ring (the b broadcast below is a
            # slow 128-descriptor replication spray; W must not sit behind
            # it -- the W path feeds the linear tail)
            w_sb = singles.tile([P, P], f32)
            nc.scalar.dma_start(w_sb, w_d[:, :])
            # bias broadcast across partitions: b_sb[p, o] = b[o]
            b_sb = singles.tile([P, F], f32)
            b_ap = b_d[:]
            nc.scalar.dma_start(
                b_sb, bass.AP(tensor=b_ap.tensor, offset=b_ap.offset, ap=[[0, P], *b_ap.ap])
            )

            # regular (HAM-visible) matmul activity during the DMA fill
            # phase: identity x identity into scratch PSUM (fp32 out, so
            # [P,4,P] f32 matches the 2KiB bank footprint of the fp16
            # transpose tiles sharing this pool slot)
            for w in range(warm // 4):
                wtp = tp_psum.tile([P, 4, P], f32, name="tp", tag="tp")
                for s in range(4):
                    nc.tensor.matmul(
                        wtp[:, s, :], ident_t, ident_t, start=True, stop=True
                    )

            # W cast fp32->fp16 on DVE (idle early); the transpose itself is
            # deferred to the flush so the in-order PE queue never stalls on
            # the W DMA ahead of the adj tile transposes
            w16 = singles.tile([P, P], agg_dt)
            nc.vector.tensor_copy(w16, w_sb)
            wt_sb = singles.tile([P, P], agg_dt)

            junk_sb = singles.tile([P, N], agg_dt)     # reduce pass scratch out
            xp_sb = singles.tile([P, NT, F], agg_dt)   # x' = d * x
            at_sb = singles.tile([P, NT, N], agg_dt)   # resident A^T
            aggt_sb = singles.tile([P, N], agg_dt)     # aggT = (A+I)x' transposed
            out_sb = singles.tile([P, NT, F], f32)
            d_all = singles.tile([P, NT], f32)         # d = (rowsum+1)^-1/2

            # one slot per accumulator: distinct tags keep all NCH tiles
            # simultaneously resident (they accumulate across the whole kernel)
            agg_ps = [
                agg_psum.tile([P, NCHUNK], f32, name=f"agg_ps{i}", tag=f"agg{i}")
                for i in range(NCH)
            ]

            # aggregation ops per chunk: 16 regular j-steps + TPC identity
            # (self-loop) steps; each ready at a known row-tile time
            agg_plan = [[] for _ in range(NCH)]
            for i in range(NCH):
                ready_i = TPC * i + TPC - 1  # chunk's AT columns complete
                for j in range(NT):
                    agg_plan[i].append((max(ready_i, j), "j", j))
                for u in range(TPC * i, TPC * (i + 1)):
                    agg_plan[i].append((max(ready_i, u), "ident", u))
            agg_emitted = [0] * NCH
            agg_pending = []  # ready steps carried across tiles (burst cap)

            def emit_agg(i, kind, idx):
                total = len(agg_plan[i])
                first = agg_emitted[i] == 0
                agg_emitted[i] += 1
                last = agg_emitted[i] == total
                if kind == "j":
                    rhs = at_sb[:, idx, NCHUNK * i : NCHUNK * (i + 1)]
                    out_ap = agg_ps[i]
                else:
                    # self-loop: aggT[:, u] += x'_u^T (= x'_u^T @ I)
                    rhs = ident_t
                    off = P * (idx - TPC * i)
                    out_ap = agg_ps[i][:, off : off + P]
                nc.tensor.matmul(
                    out_ap,
                    xp_sb[:, idx, :],
                    rhs,
                    start=first,
                    stop=last,
                )

            def fire_agg_steps(t, phase, budget):
                # the last tile's ready steps are appended (and ordered)
                # by the flush below, but its bursts still fire backlog
                # so those matmuls overlap the final transposes
                if phase == 0 and t < NT - 1:
                    for i in range(NCH):
                        for ready, kind, idx in agg_plan[i]:
                            if ready == t:
                                agg_pending.append((i, kind, idx))
                # steps touching xp_t never fire at tile t (xp_t lands
                # ~2.4us after the tile's DMA; a premature matmul would
                # stall the in-order PE queue ahead of the next tile's
                # transposes) — they carry over in the pending list
                fired = 0
                k = 0
                while fired < budget and k < len(agg_pending):
                    i, kind, idx = agg_pending[k]
                    if idx >= t:
                        k += 1
                        continue
                    agg_pending.pop(k)
                    emit_agg(i, kind, idx)
                    fired += 1

            for t in range(NT):
                if t + PREF < NT:
                    load_adj(t + PREF)
                a_c = a_tiles.pop(t)

                # rowsum: one full-width ACT pass with accumulator (the
                # wide copy output is discarded scratch; every accum path
                # runs ~1 el/cyc, and ACT's is the cheapest at 1.2 GHz),
                # then sqrt on the SAME engine — no cross-engine hop.
                # The LAST tile splits the pass DVE/ACT to shorten the
                # tail-entry degree chain (DVE is idle at that point).
                rs = small.tile([P, 1], f32, name=f"rs{t}", tag=f"rs{t}")
                if t in SPLIT:
                    rs_a = small.tile([P, 1], f32, name=f"rsh{t}", tag=f"rsh{t}")
                    nc.scalar.activation(
                        junk_sb[:, : N // 2],
                        a_c[:, : N // 2],
                        mybir.ActivationFunctionType.Copy,
                        accum_out=rs_a,
                    )
                    rs_b = small.tile([P, 1], f32, name=f"rsi{t}", tag=f"rsi{t}")
                    nc.scalar.activation(
                        junk_sb[:, N // 2 :],
                        a_c[:, N // 2 :],
                        mybir.ActivationFunctionType.Copy,
                        accum_out=rs_b,
                    )
                    nc.gpsimd.tensor_add(rs, rs_a, rs_b)
                elif t < NT - 1:
                    nc.scalar.activation(
                        junk_sb,
                        a_c,
                        mybir.ActivationFunctionType.Copy,
                        accum_out=rs,
                    )
                else:
                    rs_a = small.tile([P, 1], f32)
                    nc.vector.tensor_scalar(
                        junk_sb[:, : N // 2],
                        a_c[:, : N // 2],
                        1.0,
                        None,
                        op0=mybir.AluOpType.mult,
                        op1=mybir.AluOpType.add,  # accum reduce op
                        accum_out=rs_a,
                    )
                    rs_b = small.tile([P, 1], f32)
                    nc.scalar.activation(
                        junk_sb[:, N // 2 :],
                        a_c[:, N // 2 :],
                        mybir.ActivationFunctionType.Copy,
                        accum_out=rs_b,
                    )
                    nc.gpsimd.tensor_add(rs, rs_a, rs_b)
                # the d-chain gates the aggregation matmuls; mark it
                # high-priority so the scheduler never queues a big
                # late-input op (e.g. the next tile's rowsum pass) ahead of
                # these tiny ops in the in-order engine queues
                sq = small.tile([P, 1], f32, name=f"sq{t}", tag=f"sq{t}")
                with tc.high_priority():
                    nc.scalar.activation(
                        sq, rs, mybir.ActivationFunctionType.Sqrt, bias=1.0
                    )
                    # d-chain tail on DVE: d, then x' = d * x
                    nc.vector.reciprocal(d_all[:, t : t + 1], sq)
                    nc.vector.tensor_scalar_mul(
                        xp_sb[:, t, :], x_tiles[t], d_all[:, t : t + 1]
                    )

                # transpose all 16 blocks on the PE in 2 groups of 8
                # (one PSUM bank each); both drains on DVE, whose plain
                # PSUM copy runs in 2x mode (~0.6 ns/el vs ACT's 1x)
                for gi, g0 in enumerate(range(0, NT, 8)):
                    tp = tp_psum.tile([P, 8, P], agg_dt, name="tp", tag="tp")
                    for s in range(8):
                        j = g0 + s
                        nc.tensor.transpose(
                            tp[:, s, :], a_c[:, P * j : P * (j + 1)], ident_t
                        )
                    dst = at_sb[:, g0 : g0 + 8, P * t : P * (t + 1)]
                    if t == NT - 1 and gi == 0:
                        # last tile: split the two drains ACT/DVE so they
                        # run in parallel and the flush starts sooner
                        nc.scalar.copy(dst, tp)
                    else:
                        nc.vector.tensor_copy(dst, tp)
                    fire_agg_steps(t, gi, cap // 2 + (gi and cap % 2))

            # W^T on the PE now that all tile transposes are queued: wt
            # never gates the adj stream, only the linear tail
            wt_ps = tp_psum.tile([P, 8, P], agg_dt, name="tp", tag="tp")
            nc.tensor.transpose(wt_ps[:, 0, :], w16, ident_t)
            nc.vector.tensor_copy(wt_sb, wt_ps[:, 0, :])

            # final flush: close chunks 0..NCH-2 first (their remaining
            # steps are ready before the last chunk's AT drains land) so
            # their drain/linear/write chains overlap chunk NCH-1's burst
            t = NT - 1
            for i in range(NCH):
                for ready, kind, idx in agg_plan[i]:
                    if ready == t:
                        agg_pending.append((i, kind, idx))
            agg_pending.sort(key=lambda s: (s[0] == NCH - 1, s[0]))

            def drain_chunk(i):
                dst = aggt_sb[:, NCHUNK * i : NCHUNK * (i + 1)]
                if i % 2 == 0:
                    nc.vector.tensor_copy(dst, agg_ps[i])
                else:
                    nc.scalar.copy(dst, agg_ps[i])

            def emit_group(i):
                # linear layer for chunk i's 4 row-tiles: 4 matmuls into
                # a single PSUM bank (no slot-recycle pressure), then
                # out = d[n]*o2 + b per tile (d is per-PARTITION in o2's
                # [n, o] layout), and one 4-tile store
                # the linear output reuses chunk i's (drained) agg bank, so
                # the four groups have independent PSUM and never serialize
                # on slot recycling
                o2 = agg_psum.tile([P, NCHUNK], f32, name=f"o2_{i}", tag=f"agg{i}")
                for k in range(TPC):
                    u = TPC * i + k
                    nc.tensor.matmul(
                        o2[:, P * k : P * (k + 1)],
                        aggt_sb[:, P * u : P * (u + 1)],
                        wt_sb,
                        start=True,
                        stop=True,
                    )
                for k in range(TPC):
                    u = TPC * i + k
                    nc.vector.scalar_tensor_tensor(
                        out_sb[:, u, :],
                        o2[:, P * k : P * (k + 1)],
                        d_all[:, u : u + 1],
                        b_sb,
                        op0=mybir.AluOpType.mult,
                        op1=mybir.AluOpType.add,
                    )
                u0 = TPC * i
                dst = bass.AP(
                    tensor=o_d[:, :].tensor,
                    offset=u0 * P * F,
                    ap=[[F, P], [P * F, TPC], [1, F]],
                )
                eng = nc.sync if i % 2 == 0 else nc.scalar
                eng.dma_start(dst, out_sb[:, u0 : u0 + TPC, :])

            # flush order: (1) leftover xp-ready steps of chunks 0..2,
            # (2) their j=15 closes (gated only on xp_15, which lands
            # before tile 15's AT drains) + drains, (3) the last chunk's
            # burst, so chunks 0..2's linear/store chains overlap it
            ready_now = [s for s in agg_pending if s[2] != NT - 1]
            need_last = [s for s in agg_pending if s[2] == NT - 1]
            ready_now.sort(key=lambda s: s[0] == NCH - 1)
            need_last.sort(key=lambda s: s[0] == NCH - 1)
            for s in ready_now:
                emit_agg(*s)
            for s in need_last:
                emit_agg(*s)
            for i in range(NCH - 1):
                drain_chunk(i)
            for i in range(NCH - 1):
                emit_group(i)
            drain_chunk(NCH - 1)
            emit_group(NCH - 1)

    nc.compile()
    return nc


def get_program(agg_dtype_name="float16", bufs_a=10, warm=WARMUP, cap=AGG_CAP):
    key = (agg_dtype_name, bufs_a, warm, cap)
    if key not in _PROGRAM_CACHE:
        _PROGRAM_CACHE[key] = _build_program(agg_dtype_name, bufs_a, warm, cap)
    return _PROGRAM_CACHE[key]


def kernel(x, adj, W, b, _trace=False, _agg_dtype="float16", _pe_blocks=None,
           _trace_cores=None, _bufs_a=10, _warm=WARMUP, _cap=AGG_CAP):
    from concourse.bass_utils import run_bass_kernel_spmd

    if _trace_cores is None:
        _trace_cores = [0]
    nc = get_program(_agg_dtype, _bufs_a, _warm, _cap)
    x = np.ascontiguousarray(np.asarray(x), dtype=np.float32)
    adj = np.ascontiguousarray(np.asarray(adj), dtype=np.float32)
    W = np.ascontiguousarray(np.asarray(W), dtype=np.float32)
    b = np.ascontiguousarray(np.asarray(b), dtype=np.float32)

    in_maps = [
        {"x": x[c], "adj": adj[c], "W": W, "b": b} for c in range(B)
    ]
    res = run_bass_kernel_spmd(
        nc, in_maps, list(range(B)), trace=_trace,
        trace_cores=_trace_cores if _trace else None,
    )
    out = np.stack([res.results[c]["out"] for c in range(B)], axis=0)
    if _trace:
        return out, res
    return out

